# revision 15
# baseline (speedup 1.0000x reference)
"""5G Polar encoder (CRC11 + subchannel alloc + butterfly + interleave) on 8 trn2 cores.

The whole reference computation is GF(2)-linear in u:
    parity  = (u @ crc_gen) mod 2                       -> linear
    bits    = [u | parity] = u @ [I | crc_gen]          -> linear
    scatter x[:, info_pos] = bits                       -> column selection (linear)
    butterfly stages x ^= x[:, g[s]]                    -> linear over GF(2)
    out     = x[:, perm_out]                            -> column gather (linear)

So on the host we compose one binary matrix M [512, 1024] from the tiny index
tables (cheap uint8 ops), and the device kernel is a single fused
    y = (u @ M) mod 2
data-parallel over the batch: each of the 8 cores computes an [8192, 512] @
[512, 1024] matmul in fp8e4 with DoubleRow perf mode (exact: all values are
0/1, sums <= 512 accumulate in f32 PSUM). The mod-2 runs on the eviction
path: ACT converts PSUM f32 -> i16, DVE ANDs with 1, and the {0,1} i16
tile is DMA'd out directly (host converts to f32).

HW notes from benchmarking (slope method, marginal per 64-tile pass):
  i16out ~54us < dve ~77us < dmacast ~82us << pool ~272us (Pool copies are
  slow on HW; the tensor_scalar `mod` op and ACT `Sin` do not exist/work on
  HW, hence the integer AND path).

Roofline analysis (2026-08-09 session): the kernel is PE-bound at the fp8
DoubleRow peak. Per core: 131072 moving-column feeds (64 b-tiles x 4 MMs x
512 cols) at 1 col/cycle @ 2.4 GHz = 54.6us/pass, matching the measured
54us marginal exactly. Engine budgets per pass sit below that: ACT W1
48/64 tiles ~41us, DVE AND+W1 ~35us, out-DMA 16MB @358GB/s ~47us. A
Kronecker factorization of the polar transform (G1024 = Ga (x) Gb) cuts PE
work up to 9x on paper but requires a partition-axis transpose between
stages, which costs more than it saves on every available path (PE
permutation matmuls, DVE cross-tile XORs, per-descriptor DMA scatter).
Single-execution device time ~60us (54.6 steady + ~5us startup/drain).

The previously reported 187710ns baseline was dominated by host/axon
dispatch overhead, not device time: with fast_dispatch_compile and
device-resident chained inputs the same NEFF measures ~60.5us/exec.
Half-tile eviction (i16h: evict each 512-col psum bank separately) and
per-bank psum tiles (i16hs) both LOSE: 86/92us per pass vs 64us for the
full-tile path - doubling ACT/DVE/DMA instruction count costs far more
than the finer psum recycling saves. Full-tile i16out is the optimum.
Narrowing tensor_scalar (i16 in -> i8 out) is rejected by the walrus
verifier, and a stride-2-byte i8 view DMA of the i16 AND result overflows
a 16-bit ISA descriptor field (131072 elements) - both dead ends for
halving output bytes without extra engine work.
"""

import numpy as np
import ml_dtypes

N_CORES = 8
BS = 65536
K = 512          # u feature dim (contraction)
N = 1024         # output columns
SHARD = BS // N_CORES  # 8192 batch rows per core
P = 128
KT = K // P      # 4 k-tiles
NB = SHARD // P  # 64 batch tiles per core

FP8_NP = ml_dtypes.float8_e4m3

_nc_cache = {}


def build_M(crc_gen, info_pos, ind_gather, perm_out):
    """Compose the encoder into one GF(2) matrix M [K, N]: out = (u @ M) mod 2."""
    crc_gen = np.asarray(crc_gen)
    info_pos = np.asarray(info_pos)
    ind_gather = np.asarray(ind_gather)
    perm_out = np.asarray(perm_out)
    k, _ = crc_gen.shape
    nb, n1 = ind_gather.shape
    kp = info_pos.shape[0]
    C = (crc_gen.astype(np.int64) & 1).astype(np.uint8)
    B = np.concatenate([np.eye(k, dtype=np.uint8), C], axis=1)  # [k, kp]
    # scatter bits into columns; duplicate indices: last write wins (matches
    # jax/numpy .at[].set application order)
    col_src = np.full(n1, -1, np.int64)
    col_src[info_pos] = np.arange(kp)
    A = np.zeros((k, n1), np.uint8)
    valid = col_src >= 0
    A[:, valid] = B[:, col_src[valid]]
    for s in range(nb):
        A = A ^ A[:, ind_gather[s]]
    return A[:, perm_out]  # [k, n]


def _build_nc(reps=1, do_mm=True, do_evict=True, evict="pool",
              w1_act=64, w3_dve=0, ev_stage=3, u_chunks=1, wbufs=4,
              ks_outer=False, do_in=True, n_tiles=None):
    """evict modes:
    - "pool":    ACT f32->i16, DVE AND, Pool narrow i16->i8, DMA i8
    - "dve":     ACT f32->i16, DVE AND, DVE narrow i16->i8, DMA i8
    - "i16out":  ACT f32->i16, DVE AND, DMA out i16 (host takes low bits)
    - "dmacast": ACT f32->i16, DVE AND, gpsimd casting DMA i16->i8
    - "split":   W1 on ACT for w1_act tiles/64 else DVE; AND on DVE;
                 narrow on DVE for w3_dve tiles/64 else Pool; DMA i8
    """
    import concourse.tile as tile
    from concourse import bacc, mybir

    nc = bacc.Bacc("TRN2", target_bir_lowering=False, debug=False)
    fp8 = mybir.dt.float8e4
    f32 = mybir.dt.float32
    i16 = mybir.dt.int16
    i8 = mybir.dt.int8
    DR = mybir.MatmulPerfMode.DoubleRow

    # k-major 3D layouts: [p, ks, free] with global k = ks*128 + p (both
    # operands use the same mapping, so the contraction is correct).
    uT = nc.declare_dram_parameter("uT", [P, KT, SHARD], fp8, isOutput=False)
    mat = nc.declare_dram_parameter("mat", [P, KT, N], fp8, isOutput=False)
    y_dt = i16 if evict in ("i16out", "i16h", "i16hs") else i8
    y = nc.declare_dram_parameter("y", [SHARD, N], y_dt, isOutput=True)
    if n_tiles is None:
        n_tiles = NB

    with tile.TileContext(nc) as tc:
        with (
            tc.tile_pool(name="consts", bufs=1) as cpool,
            tc.tile_pool(name="work", bufs=wbufs) as wpool,
            tc.tile_pool(name="outs", bufs=4) as opool,
            tc.tile_pool(name="psum", bufs=4, space="PSUM") as ppool,
        ):
            mt = cpool.tile([P, KT, N], fp8, tag="mt")
            if do_in:
                nc.sync.dma_start(mt[:], mat[:])
            # chunk the big u load along batch so the first b-tile's matmuls
            # start after ~1/u_chunks of the 4MB has landed
            CW = SHARD // u_chunks
            uts = []
            for c in range(u_chunks):
                ut_c = cpool.tile([P, KT, CW], fp8, tag=f"ut{c}", name=f"ut{c}")
                if do_in:
                    nc.sync.dma_start(ut_c[:], uT[:, :, c * CW:(c + 1) * CW])
                uts.append(ut_c)
            ot_shared = None
            if evict == "outonly":
                ot_shared = cpool.tile([P, N], i8, tag="ot_shared")
                nc.any.memset(ot_shared[:], 0)
            ps_shared = None
            if not do_mm:
                ps_shared = ppool.tile([P, N], f32, tag="ps_shared")
                for h in range(2):
                    nc.tensor.matmul(
                        ps_shared[:, h * 512:(h + 1) * 512],
                        uts[0][:, 0:2, 0:P],
                        mt[:, 0:2, h * 512:(h + 1) * 512],
                        start=True, stop=True, perf_mode=DR,
                    )
            for i, b in enumerate(
                [b for _ in range(reps) for b in range(n_tiles)]
            ):
                if do_mm:
                    if evict == "i16hs":
                        # one psum tile per 512-col bank: banks recycle
                        # independently instead of in 2-bank pairs
                        ps_halves = [
                            ppool.tile([P, 512], f32, tag=f"psh{h}",
                                       name=f"psh{h}")
                            for h in range(2)
                        ]
                        ps = None
                    else:
                        ps = ppool.tile([P, N], f32, tag="ps", name="ps")
                else:
                    ps = ps_shared
                t16 = wpool.tile([P, N], i16, tag="t16")
                a16 = wpool.tile([P, N], i16, tag="a16")
                ot = opool.tile([P, N], i8, tag="ot")
                if do_mm:
                    ut = uts[(b * P) // CW]
                    boff = (b * P) % CW
                    loop = (
                        [(h, ks) for ks in range(0, KT, 2) for h in range(2)]
                        if ks_outer else
                        [(h, ks) for h in range(2) for ks in range(0, KT, 2)]
                    )
                    for h, ks in loop:
                        dst = (ps_halves[h][:] if evict == "i16hs"
                               else ps[:, h * 512:(h + 1) * 512])
                        nc.tensor.matmul(
                            dst,
                            ut[:, ks:ks + 2, boff:boff + P],
                            mt[:, ks:ks + 2, h * 512:(h + 1) * 512],
                            start=(ks == 0),
                            stop=(ks == KT - 2),
                            perf_mode=DR,
                            skip_group_check=ks_outer,
                        )
                if do_evict:
                    if evict == "i16hs":
                        for h in range(2):
                            sl = slice(h * 512, (h + 1) * 512)
                            th = wpool.tile([P, 512], i16, tag=f"th{h}")
                            ah = wpool.tile([P, 512], i16, tag=f"ah{h}")
                            if h == 0 or (i % 4) == 3:
                                nc.scalar.activation(
                                    th[:], ps_halves[h][:],
                                    mybir.ActivationFunctionType.Copy,
                                )
                            else:
                                nc.vector.tensor_copy(th[:], ps_halves[h][:])
                            nc.vector.tensor_scalar(
                                ah[:], th[:], 1, None,
                                mybir.AluOpType.bitwise_and,
                            )
                            nc.sync.dma_start(
                                y[b * P:(b + 1) * P, sl], ah[:])
                        continue
                    if evict == "i16h":
                        # half-tile eviction: each 512-col psum bank is
                        # converted+ANDed+stored as soon as its ks-group
                        # finishes, freeing banks earlier for the PE.
                        # W1 split: h0 always ACT; h1 ACT every 4th tile.
                        for h in range(2):
                            sl = slice(h * 512, (h + 1) * 512)
                            th = wpool.tile([P, 512], i16, tag=f"th{h}")
                            ah = wpool.tile([P, 512], i16, tag=f"ah{h}")
                            if h == 0 or (i % 4) == 3:
                                nc.scalar.activation(
                                    th[:], ps[:, sl],
                                    mybir.ActivationFunctionType.Copy,
                                )
                            else:
                                nc.vector.tensor_copy(th[:], ps[:, sl])
                            nc.vector.tensor_scalar(
                                ah[:], th[:], 1, None,
                                mybir.AluOpType.bitwise_and,
                            )
                            nc.sync.dma_start(
                                y[b * P:(b + 1) * P, sl], ah[:])
                        continue
                    if evict == "peprobe":
                        # consume 1 element of the psum so the bank recycles
                        # without real eviction work: isolates PE rate
                        tp = wpool.tile([P, 1], i16, tag="tp")
                        nc.scalar.activation(
                            tp[:], ps[:, 0:1],
                            mybir.ActivationFunctionType.Copy,
                        )
                        continue
                    if evict == "outonly":
                        nc.sync.dma_start(y[b * P:(b + 1) * P, :], ot_shared[:])
                        continue
                    # W1: PSUM f32 -> i16
                    if ev_stage >= 1:
                        if evict == "w1dve" or (i % NB) >= w1_act:
                            nc.vector.tensor_copy(t16[:], ps[:])
                        else:
                            nc.scalar.activation(
                                t16[:], ps[:],
                                mybir.ActivationFunctionType.Copy,
                            )
                    # W2: AND with 1
                    if ev_stage >= 2:
                        if evict == "i8sb":
                            # AND in i16, then DMA only the low bytes
                            # (stride-2 i8 view) -> i8 output, no extra op
                            nc.vector.tensor_scalar(
                                a16[:], t16[:], 1, None,
                                mybir.AluOpType.bitwise_and,
                            )
                            a8 = a16[:].bitcast(i8)
                            nc.sync.dma_start(
                                y[b * P:(b + 1) * P, :], a8[:, 0::2])
                            continue
                        if evict == "i8and":
                            # narrowing AND: i16 in -> i8 out in one DVE op
                            nc.vector.tensor_scalar(
                                ot[:], t16[:], 1, None,
                                mybir.AluOpType.bitwise_and,
                            )
                            nc.sync.dma_start(y[b * P:(b + 1) * P, :], ot[:])
                            continue
                        nc.vector.tensor_scalar(
                            a16[:], t16[:], 1, None,
                            mybir.AluOpType.bitwise_and,
                        )
                    # W3 + output DMA
                    if ev_stage < 3:
                        continue
                    if evict == "i16out":
                        nc.sync.dma_start(y[b * P:(b + 1) * P, :], a16[:])
                    elif evict in ("dmacast", "w1dve"):
                        nc.gpsimd.dma_start(y[b * P:(b + 1) * P, :], a16[:])
                    else:
                        if evict == "dve" or (
                            evict == "split" and (i % NB) < w3_dve
                        ):
                            nc.vector.tensor_copy(ot[:], a16[:])
                        else:
                            nc.gpsimd.tensor_copy(ot[:], a16[:])
                        nc.sync.dma_start(y[b * P:(b + 1) * P, :], ot[:])
    nc.compile()
    return nc


EVICT = "i16out"
W1_ACT = 48      # 48/64 PSUM->i16 converts on ACT, 16/64 on DVE
U_CHUNKS = 8     # input u loaded in 8 chunks so matmuls start early
WBUFS = 6
KS_OUTER = True  # k-pair outer loop: one LDWEIGHTS serves both psum halves


def get_nc(reps=1):
    key = (reps, EVICT, W1_ACT, U_CHUNKS, WBUFS, KS_OUTER)
    if key not in _nc_cache:
        _nc_cache[key] = _build_nc(reps, evict=EVICT, w1_act=W1_ACT,
                                   u_chunks=U_CHUNKS, wbufs=WBUFS,
                                   ks_outer=KS_OUTER)
    return _nc_cache[key]


def _to_k_major(a_km, free):
    """[K, free] -> [P, KT, free] with k = ks*128 + p."""
    return np.ascontiguousarray(
        a_km.reshape(KT, P, free).transpose(1, 0, 2)
    )


def make_in_maps(u, M):
    u8 = np.asarray(u).astype(FP8_NP)
    m8 = np.asarray(M).astype(FP8_NP)
    mat3 = _to_k_major(m8, N)
    in_maps = []
    for i in range(N_CORES):
        uT_i = np.ascontiguousarray(u8[i * SHARD:(i + 1) * SHARD, :].T)
        in_maps.append({"uT": _to_k_major(uT_i, SHARD), "mat": mat3})
    return in_maps


def kernel(u, crc_gen, info_pos, ind_gather, perm_out):
    from concourse.bass_utils import run_bass_kernel_spmd

    M = build_M(crc_gen, info_pos, ind_gather, perm_out)
    in_maps = make_in_maps(u, M)
    nc = get_nc()
    res = run_bass_kernel_spmd(nc, in_maps, core_ids=list(range(N_CORES)))
    out = np.concatenate(
        [np.asarray(r["y"]).astype(np.float32) for r in res.results], axis=0
    )
    return out

